# revision 26
# baseline (speedup 1.0000x reference)
"""Trainium2 Bass kernel for nn_Attention_72103910965317 (v2).

Multi-head self-attention block (4 heads, head_dim 32, N=4096 tokens/batch,
c=128 channels) over inputs x:[4,64,64,128].

Sharding: 8 cores; core c handles batch c//2 and heads {2*(c%2), 2*(c%2)+1}
(data-parallel over batch x tensor-parallel over heads). Each core computes
per-head attention + its heads' slice of the output projection UNNORMALIZED
(plus the per-head softmax row-sums); the host applies the 1/rowsum
normalization, sums the two per-core partials per batch, and adds b_out.

v2 changes vs v1 (301.9us):
  - The softmax exp (33.5M elems/core, previously 248us of serial ScalarE
    time = the critical path) is SPLIT between ScalarE (true Exp activation)
    and the DVE via a custom 8-stage DVE op (EXP16_BITS_ANT): a floor-based
    Schraudolph bit-trick with quadratic mantissa correction producing fp16
    bits in one 1x-rate pass (max rel err ~0.2%). Both paths emit 0.5*e^s
    (the 0.5 is a bit-trick artifact; softmax normalization cancels it).
  - Scores matmuls feed the bit-trick directly: wq is pre-scaled by
    KAPPA=1024*log2(e) so PSUM holds S' = KAPPA*s; ScalarE path un-scales
    via the activation's free scale/bias slots.
  - Softmax normalization moved to the HOST: the kernel ships per-head
    unnormalized projections (fp16) + row sums; host does
    y = yh0*r0 + yh1*r1 + b_out. Kills the v1 DRAM-bounce/reciprocal/
    per-tile normalize chain on the DVE.
  - AV accumulates both heads into ONE PSUM bank (partition strips 0:33 /
    64:97 via col tile_position), evacuated by a single ScalarE copy.
"""

import os
import sys
import contextlib

for _p in ("/opt/trn_rl_repo", "/root/.axon_site/_ro/trn_rl_repo"):
    if os.path.isdir(_p) and _p not in sys.path:
        sys.path.insert(0, _p)

import numpy as np

import concourse.bass as bass
import concourse.tile as tile
from concourse import bacc, mybir
from concourse.bass_utils import run_bass_kernel_spmd

dt = mybir.dt
AF = mybir.ActivationFunctionType

N_CORES = 8
B, HGT, WID, C = 4, 64, 64, 128
N = HGT * WID          # 4096 tokens per batch
HEADS, D = 4, 32       # heads, head dim
SCALE = D ** -0.5
NT = N // 128          # 32 j-tiles / i-tiles
NIC = N // 512         # 8 i-chunks
VROW = 2 * (D + 1)     # 66: V_aug row for both heads [V_h0|1|V_h1|1]

# ---- custom DVE exp constants (see fit in problem notes) ----
KAPPA = 1024.0 * np.log2(np.e)          # 1477.3199...
EXP_H = 14839.92186188657               # s0: octave-alignment offset
EXP_M = 1.5 * 2.0 ** 33                 # s1: round-to-1024 magic
EXP_ALPHA = 0.0003292511551447068       # imm2: quadratic mantissa coeff
EXP_G = -591.0718312168698 + 1024.0     # C3 (via in1): bits bias (+1024
                                        # doubles the bit-trick output to
                                        # 1.0*e^s, matching plain ScalarE exp)

_CACHE = {}


def _register_exp16_op():
    """Define + register the EXP16_BITS_ANT custom DVE op (idempotent).

    bits = sq(F)*alpha + T + g, T = Src0 + H, F = T - ((T+M)-M)
    out(int16) bit-cast to fp16 gives 0.5*exp(Src0/KAPPA) to ~0.2%.
    """
    from concourse import dve_ops as dops
    from concourse.dve_spec import (
        Spec, Src0, C0, C1, C2, C3, _spill_c3_to_src1, sq, lower,
    )
    from concourse.dve_uop import DveOpSpec

    name = "EXP16_BITS_ANT"
    if name in dops._SUB_OPCODE_FOR_NAME:
        return next(op for op in dops.OPS if op.name == name)

    _T = Src0 + C0
    _u = _T + C1
    _K = _u - C1
    _F = _T - _K
    _body = (sq(_F) * C2 + _T) + C3

    def _ref(in0, in1, s0, s1, imm2):
        f32 = np.float32
        t = (in0.astype(f32) + f32(s0)).astype(f32)
        u = (t + f32(s1)).astype(f32)
        k = (u - f32(s1)).astype(f32)
        fq = (t - k).astype(f32)
        g = np.asarray(in1, f32).reshape(-1, 1)
        return ((fq * fq) * f32(imm2) + t + g).astype(f32)

    spec = Spec(body=_spill_c3_to_src1(_body), reference=_ref)

    row = dops._CUSTOM_DVE_ROW_BASE + len(dops.OPS)
    assert row < 0x20
    dops._SUB_OPCODE_FOR_NAME[name] = row

    shas = {}
    for ver in ("v3", "v4"):
        try:
            uops = lower(spec, ver=ver)
            shas[ver] = DveOpSpec(
                name=name, opcode=row, uops=uops, rd1_en=True
            ).sha(ver)
        except Exception:
            pass

    op = dops.DveOp(name, spec, subdim=False, uops_sha=shas)
    dops.OPS.append(op)
    dops.CUSTOM_DVE_SPECS[name] = spec
    return op


EXP16_OP = _register_exp16_op()


def _build_program():
    nc = bacc.Bacc("TRN2", target_bir_lowering=False, debug=False,
                   enable_asserts=True, num_devices=N_CORES)

    # ---- per-core DRAM I/O ----
    xt_d = nc.dram_tensor("xt", [128, N], dt.float16, kind="ExternalInput").ap()
    wq0_d = nc.dram_tensor("wq0", [128, 64], dt.float16, kind="ExternalInput").ap()
    wq1_d = nc.dram_tensor("wq1", [128, 64], dt.float16, kind="ExternalInput").ap()
    wk0_d = nc.dram_tensor("wk0", [128, 32], dt.float16, kind="ExternalInput").ap()
    wk1_d = nc.dram_tensor("wk1", [128, 32], dt.float16, kind="ExternalInput").ap()
    wv_d = nc.dram_tensor("wv", [128, 64], dt.float16, kind="ExternalInput").ap()
    wo_d = nc.dram_tensor("wo", [128, 128], dt.float16, kind="ExternalInput").ap()
    # per-head unnormalized projections + row sums (host normalizes)
    y0_d = nc.dram_tensor("y0", [N, 128], dt.float16, kind="ExternalOutput").ap()
    y1_d = nc.dram_tensor("y1", [N, 128], dt.float16, kind="ExternalOutput").ap()
    r_d = nc.dram_tensor("r", [2, N], dt.float16, kind="ExternalOutput").ap()

    ctx = contextlib.ExitStack()
    with tile.TileContext(nc) as tc, ctx:
        # ---- persistent SBUF ----
        per = ctx.enter_context(tc.tile_pool(name="per", bufs=1))
        wq = [per.tile([128, 64], dt.float16, tag=f"wq{h}", name=f"wq{h}")
              for h in range(2)]
        wk = [per.tile([128, 32], dt.float16, tag=f"wk{h}", name=f"wk{h}")
              for h in range(2)]
        wv = per.tile([128, 64], dt.float16)
        wo = per.tile([128, 128], dt.float16)
        # weight + xT DMAs are triggered from the Scalar/GpSimd sequencers:
        # Sync spends ~14us on preamble DIRECT2D writes at kernel start, so
        # DMAs queued there don't even begin until ~9us in. Scalar's preamble
        # is ~2.6us and GpSimd's ~5us; route the early-needed data there.
        nc.scalar.dma_start(wk[0][:], wk0_d[:])
        nc.scalar.dma_start(wq[0][:], wq0_d[:])
        nc.gpsimd.dma_start(wv[:], wv_d[:])
        nc.gpsimd.dma_start(wk[1][:], wk1_d[:])
        nc.gpsimd.dma_start(wq[1][:], wq1_d[:])
        nc.gpsimd.dma_start(wo[:], wo_d[:])
        # xT split into 3 column-chunk tiles (12/12/8 token-tiles)
        XC = (1536, 1536, 1024)
        xt_c = [per.tile([128, XC[ci]], dt.float16, tag=f"xt{ci}",
                         name=f"xt{ci}") for ci in range(3)]
        for ci, eng in enumerate((nc.scalar, nc.gpsimd, nc.sync)):
            eng.dma_start(xt_c[ci][:], xt_d[:, 1536 * ci:1536 * ci + XC[ci]])
        warm = per.tile([1, 8], dt.float32)
        nc.scalar.activation(warm[:], wv[0:1, 0:8], AF.Exp)
        # C3 constant tile for the custom DVE exp
        gconst = per.tile([128, 1], dt.float32, tag="gconst", name="gconst")
        nc.vector.memset(gconst[:], EXP_G)

        # Q^T replicated x2, split into 3 column-chunk tiles per head so
        # scores only RAW-depend on the chunk covering their i-range;
        # K^T block layout: 2-row groups [64, 128*G] (16 groups of 2 j-tiles
        # -> [128,1024] score tiles, 2 PSUM banks each, 3-deep rotation)
        qt = [[per.tile([64, 1536 if q < 2 else 1024], dt.float16,
                        tag=f"qt{h}_{q}", name=f"qt{h}_{q}") for q in range(3)]
              for h in range(2)]
        kt = [[per.tile([64, w], dt.float16, tag=f"kt{h}_{ci}",
                        name=f"kt{h}_{ci}")
               for ci, w in enumerate((768, 768, 512))] for h in range(2)]
        # V_aug for both heads: 4 tiles of 8 j-tiles [128, 8*66] fp16
        # (ones pre-set by memset; split for finer RAW dependencies)
        vsb = [per.tile([128, 8 * VROW], dt.float16, tag=f"v{q}",
                        name=f"vsb{q}") for q in range(4)]
        for q in range(4):
            nc.gpsimd.memset(vsb[q][:], 1.0)

        # ---- PSUM pools: 3x [128,1024] score slots (6 banks) + 2 "o" ----
        ps_s = ctx.enter_context(tc.tile_pool(name="ps_s", bufs=3, space="PSUM"))
        ps_o = ctx.enter_context(tc.tile_pool(name="ps_o", bufs=2, space="PSUM"))

        sb_p = ctx.enter_context(tc.tile_pool(name="sb_p", bufs=4))
        sb_t = ctx.enter_context(tc.tile_pool(name="sb_t", bufs=2))
        sb_y = ctx.enter_context(tc.tile_pool(name="sb_y", bufs=4))

        # chunk views as [p, token-tile, 128]
        xt3c = [xc.rearrange("p (t jj) -> p t jj", jj=128) for xc in xt_c]

        # ---- prologue projections ----
        def emit_v_round(q):
            pv = ps_s.tile([128, 512], dt.float32, tag="s", name="pv")
            for k in range(8):
                jt = 8 * q + k
                nc.tensor.matmul(pv[:, 64 * k:64 * k + 64],
                                 xt3c[jt // 12][:, jt % 12, :],
                                 wv[:], start=True, stop=True)
            nc.vector.tensor_copy(
                vsb[q][:].rearrange(
                    "p (t a b) -> p t a b", t=8, b=33)[:, :, :, 0:32],
                pv[:].rearrange("p (t a b) -> p t a b", t=8, b=32))

        def emit_kt(h, ci):
            # kt[h][ci][32r+d, 128G'+jj] = K_h[(2(base+G')+r)*128+jj, d]
            cnt = 6 if ci < 2 else 4
            pk = ps_s.tile([128, 768], dt.float32, tag="s", name="pk")
            for r in range(2):
                for p0 in range(0, cnt, 4):   # <=512 moving-free per matmul
                    pc = min(4, cnt - p0)
                    rhs = xt3c[ci][:, 2 * p0 + r:
                                   2 * (p0 + pc - 1) + r + 1:2, :]
                    nc.tensor.matmul(
                        pk[32 * r:32 * r + 32, 128 * p0:128 * (p0 + pc)],
                        wk[h][:], rhs, start=True, stop=True,
                        tile_position=(0, 32 * r))
            nc.vector.tensor_copy(kt[h][ci][0:64, :],
                                  pk[0:64, 0:cnt * 128])

        def emit_qt(h, q):
            nch = 3 if q < 2 else 2
            for k in range(nch):
                pq = ps_s.tile([128, 512], dt.float32, tag="s", name="pq")
                nc.tensor.matmul(pq[0:64, :], wq[h][:],
                                 xt_c[q][:, 512 * k:512 * (k + 1)],
                                 start=True, stop=True)
                nc.vector.tensor_copy(qt[h][q][0:64, 512 * k:512 * (k + 1)],
                                      pq[0:64, :])

        # minimal prologue upfront; the rest is interleaved into ic=0's
        # groups (emitted just before the group's scores) so the PE starts
        # the scores/exp pipeline ~15us earlier instead of idling behind
        # DMA-gated projection matmuls.
        emit_kt(0, 0)
        emit_qt(0, 0)
        emit_kt(1, 0)
        emit_qt(1, 0)
        emit_v_round(0)
        prologue_sched = {
            1: [lambda: emit_kt(0, 1), lambda: emit_kt(1, 1)],
            2: [lambda: emit_v_round(1)],
            4: [lambda: emit_kt(0, 2), lambda: emit_kt(1, 2)],
            6: [lambda: emit_qt(0, 1)],
            8: [lambda: emit_v_round(2)],
            9: [lambda: emit_qt(1, 1)],
            11: [lambda: emit_qt(0, 2)],
            12: [lambda: emit_v_round(3)],
            13: [lambda: emit_qt(1, 2)],
        }

        # ---- main loop ----
        groups = [(g, 2) for g in range(16)]

        def emit_proj_h(ic, ot, h):
            # output projection (unnormalized); yh fp16 shipped to host
            pm = ps_s.tile([128, 512], dt.float32, tag="s", name="pm")
            for t4 in range(4):
                nc.tensor.matmul(pm[:, 128 * t4:128 * (t4 + 1)],
                                 ot[64 * h:64 * h + 32,
                                    t4 * 128:(t4 + 1) * 128],
                                 wo[64 * h:64 * h + 32, :],
                                 start=True, stop=True,
                                 tile_position=(64 * h, 0))
            yh = sb_y.tile([128, 512], dt.float16, tag=f"yh{h}",
                           name=f"yh{h}")
            eng = nc.scalar if h == 0 else nc.vector
            if h == 0:
                nc.scalar.activation(yh[:], pm[:], AF.Copy)
            else:
                nc.vector.tensor_copy(yh[:], pm[:])
            yd = y0_d if h == 0 else y1_d
            (nc.sync if h == 0 else nc.gpsimd).dma_start(
                yd[ic * 512:(ic + 1) * 512, :].rearrange(
                    "(t p) c -> p t c", p=128),
                yh[:].rearrange("p (t c) -> p t c", c=128))

        def emit_av(ic, g, nt_, po, pts):
            # AV for both heads, interleaved by j-tile; both heads accumulate
            # into ONE PSUM bank at partition strips 0:33 / 64:97.
            for r in range(nt_):
                jt = 2 * g + r
                for h in range(2):
                    nc.tensor.matmul(
                        po[64 * h:64 * h + 33, :],
                        vsb[jt // 8][:, (jt % 8) * VROW + 33 * h:
                                     (jt % 8) * VROW + 33 * h + 33],
                        pts[h][:, 512 * r:512 * (r + 1)],
                        start=(jt == 0),
                        stop=(jt == NT - 1),
                        tile_position=(0, 64 * h),
                        skip_group_check=True)

        def emit_epilogue(ic, po):
            # one evacuation copy for both heads' out^T strips + sum rows;
            # ship the raw row sums (rows 32 / 96) to DRAM for the host.
            ot = sb_t.tile([128, 512], dt.float16, tag="ot")
            nc.scalar.activation(ot[:], po[:], AF.Copy)
            nc.sync.dma_start(r_d[0:1, ic * 512:(ic + 1) * 512],
                              ot[32:33, :])
            nc.gpsimd.dma_start(r_d[1:2, ic * 512:(ic + 1) * 512],
                                ot[96:97, :])
            return ot

        # exp engine assignment: per (g, h) -> 'S' (ScalarE true exp) or
        # 'V' (DVE custom bit-trick exp). ~18:14 split tuned for balance.
        def exp_engine(g, h):
            if h == 0:
                return 'S'
            return 'S' if g in (5, 11) else 'V'

        def emit_exp(eng, pt_ap, ps_ap):
            if eng == 'S':
                nc.scalar.activation(pt_ap, ps_ap, AF.Exp,
                                     scale=1.0 / KAPPA)
            else:
                nc.vector._custom_dve(
                    EXP16_OP, out=pt_ap.bitcast(dt.int16), in0=ps_ap,
                    s0=EXP_H, s1=EXP_M, imm2=EXP_ALPHA, in1=gconst[:])

        # flat software pipeline over (ic, g) steps: scores/exp run TWO
        # groups ahead of AV so the in-order PE stream never stalls waiting
        # for an exp that was issued in the immediately preceding step.
        prev_proj = None
        pend_av = []            # deque of (ic, g, nt_, po, pts), depth 2
        po = None

        def retire_av():
            item = pend_av.pop(0)
            emit_av(*item)
            if item[1] == 15:   # finished that i-chunk's AV
                return (item[0], emit_epilogue(item[0], item[3]))
            return None

        for ic in range(NIC):
            for g, nt_ in groups:
                if ic == 0 and g in prologue_sched:
                    for fn in prologue_sched[g]:
                        fn()
                if g == 0:
                    po = ps_o.tile([128, 512], dt.float32, tag="o",
                                   name="po")
                if g == 4 and prev_proj is not None:
                    emit_proj_h(*prev_proj, 0)
                if g == 8 and prev_proj is not None:
                    emit_proj_h(*prev_proj, 1)
                    prev_proj = None
                ci = g // 6
                pts = []
                for h in range(2):
                    ps = ps_s.tile([128, 1024], dt.float32, tag="s")
                    for r in range(nt_):
                        nc.tensor.matmul(
                            ps[:, 512 * r:512 * (r + 1)],
                            kt[h][ci][32 * r:32 * r + 32,
                                      (g - 6 * ci) * 128:
                                      (g - 6 * ci + 1) * 128],
                            qt[h][ic // 3][32 * r:32 * r + 32,
                                           (ic % 3) * 512:(ic % 3 + 1) * 512],
                            start=True, stop=True, tile_position=(32 * r, 0))
                    pt = sb_p.tile([128, nt_ * 512], dt.float16, tag=f"p{h}")
                    emit_exp(exp_engine(g, h), pt[:], ps[:, 0:nt_ * 512])
                    pts.append(pt)
                depth = 1 if (ic == NIC - 1 and g >= 13) else 2
                while len(pend_av) >= depth:
                    r_ = retire_av()
                    if r_ is not None:
                        prev_proj = r_
                pend_av.append((ic, g, nt_, po, pts))

        while pend_av:
            r_ = retire_av()
            if r_ is not None:
                prev_proj = r_
        emit_proj_h(*prev_proj, 0)
        emit_proj_h(*prev_proj, 1)

    nc.compile()
    return nc


def _host_prep(x, w_qkv, w_out):
    """Build per-core input maps."""
    xf = np.asarray(x, dtype=np.float32).reshape(B, N, C)
    wq_all = np.asarray(w_qkv[:, 0:128], dtype=np.float32)
    wk_all = np.asarray(w_qkv[:, 128:256], dtype=np.float32)
    wv_all = np.asarray(w_qkv[:, 256:384], dtype=np.float32)
    wo_all = np.asarray(w_out, dtype=np.float32)

    xts = [np.ascontiguousarray(xf[b].T).astype(np.float16) for b in range(B)]
    qscale = SCALE * KAPPA

    in_maps = []
    for c in range(N_CORES):
        b = c // 2
        hp = (c % 2) * 2
        wo = np.zeros((128, 128), dtype=np.float16)
        wo[0:32] = wo_all[32 * hp:32 * hp + 32, :]
        wo[64:96] = wo_all[32 * hp + 32:32 * hp + 64, :]
        m = {
            "xt": xts[b],
            "wq0": np.tile(wq_all[:, 32 * hp:32 * hp + 32] * qscale,
                           (1, 2)).astype(np.float16),
            "wq1": np.tile(wq_all[:, 32 * hp + 32:32 * hp + 64] * qscale,
                           (1, 2)).astype(np.float16),
            "wk0": wk_all[:, 32 * hp:32 * hp + 32].astype(np.float16),
            "wk1": wk_all[:, 32 * hp + 32:32 * hp + 64].astype(np.float16),
            "wv": wv_all[:, 32 * hp:32 * hp + 64].astype(np.float16),
            "wo": wo,
        }
        in_maps.append(m)
    return in_maps


def kernel(x, w_qkv, w_out, b_out, _trace=False, _tmpdir=None):
    if "nc" not in _CACHE:
        _CACHE["nc"] = _build_program()
    nc = _CACHE["nc"]

    in_maps = _host_prep(x, w_qkv, w_out)
    res = run_bass_kernel_spmd(nc, in_maps, core_ids=list(range(N_CORES)),
                               trace=_trace, tmpdir=_tmpdir)
    _CACHE["last_result"] = res

    b_out_f = np.asarray(b_out, dtype=np.float32)
    y = np.empty((B, N, C), dtype=np.float32)
    for b in range(B):
        acc = np.zeros((N, C), dtype=np.float32)
        for c in (2 * b, 2 * b + 1):
            rc = res.results[c]
            for h in range(2):
                yh = rc["y0" if h == 0 else "y1"].astype(np.float32)
                rs = rc["r"][h].astype(np.float32)
                acc += yh * (1.0 / rs)[:, None]
        y[b] = acc + b_out_f
    return y.reshape(B, HGT, WID, C)


# revision 27
# speedup vs baseline: 1.1275x; 1.1275x over previous
"""Trainium2 Bass kernel for nn_Attention_72103910965317 (v2).

Multi-head self-attention block (4 heads, head_dim 32, N=4096 tokens/batch,
c=128 channels) over inputs x:[4,64,64,128].

Sharding: 8 cores; core c handles batch c//2 and heads {2*(c%2), 2*(c%2)+1}
(data-parallel over batch x tensor-parallel over heads). Each core computes
per-head attention + its heads' slice of the output projection UNNORMALIZED
(plus the per-head softmax row-sums); the host applies the 1/rowsum
normalization, sums the two per-core partials per batch, and adds b_out.

v2 changes vs v1 (301.9us):
  - The softmax exp (33.5M elems/core, previously 248us of serial ScalarE
    time = the critical path) is SPLIT between ScalarE (true Exp activation)
    and the DVE via a custom 8-stage DVE op (EXP16_BITS_ANT): a floor-based
    Schraudolph bit-trick with quadratic mantissa correction producing fp16
    bits in one 1x-rate pass (max rel err ~0.2%). Both paths emit 0.5*e^s
    (the 0.5 is a bit-trick artifact; softmax normalization cancels it).
  - Scores matmuls feed the bit-trick directly: wq is pre-scaled by
    KAPPA=1024*log2(e) so PSUM holds S' = KAPPA*s; ScalarE path un-scales
    via the activation's free scale/bias slots.
  - Softmax normalization moved to the HOST: the kernel ships per-head
    unnormalized projections (fp16) + row sums; host does
    y = yh0*r0 + yh1*r1 + b_out. Kills the v1 DRAM-bounce/reciprocal/
    per-tile normalize chain on the DVE.
  - AV accumulates both heads into ONE PSUM bank (partition strips 0:33 /
    64:97 via col tile_position), evacuated by a single ScalarE copy.
"""

import os
import sys
import contextlib

for _p in ("/opt/trn_rl_repo", "/root/.axon_site/_ro/trn_rl_repo"):
    if os.path.isdir(_p) and _p not in sys.path:
        sys.path.insert(0, _p)

import numpy as np

import concourse.bass as bass
import concourse.tile as tile
from concourse import bacc, mybir
from concourse.bass_utils import run_bass_kernel_spmd

dt = mybir.dt
AF = mybir.ActivationFunctionType

N_CORES = 8
B, HGT, WID, C = 4, 64, 64, 128
N = HGT * WID          # 4096 tokens per batch
HEADS, D = 4, 32       # heads, head dim
SCALE = D ** -0.5
NT = N // 128          # 32 j-tiles / i-tiles
NIC = N // 512         # 8 i-chunks
VROW = 2 * (D + 1)     # 66: V_aug row for both heads [V_h0|1|V_h1|1]

# ---- custom DVE exp constants (see fit in problem notes) ----
KAPPA = 1024.0 * np.log2(np.e)          # 1477.3199...
EXP_H = 14839.92186188657               # s0: octave-alignment offset
EXP_M = 1.5 * 2.0 ** 33                 # s1: round-to-1024 magic
EXP_ALPHA = 0.0003292511551447068       # imm2: quadratic mantissa coeff
EXP_G = -591.0718312168698 + 1024.0     # C3 (via in1): bits bias (+1024
                                        # doubles the bit-trick output to
                                        # 1.0*e^s, matching plain ScalarE exp)

_CACHE = {}


def _register_exp16_op():
    """Define + register the EXP16_BITS_ANT custom DVE op (idempotent).

    bits = sq(F)*alpha + T + g, T = Src0 + H, F = T - ((T+M)-M)
    out(int16) bit-cast to fp16 gives 0.5*exp(Src0/KAPPA) to ~0.2%.
    """
    from concourse import dve_ops as dops
    from concourse.dve_spec import (
        Spec, Src0, C0, C1, C2, C3, _spill_c3_to_src1, sq, lower,
    )
    from concourse.dve_uop import DveOpSpec

    name = "EXP16_BITS_ANT"
    if name in dops._SUB_OPCODE_FOR_NAME:
        return next(op for op in dops.OPS if op.name == name)

    _T = Src0 + C0
    _u = _T + C1
    _K = _u - C1
    _F = _T - _K
    _body = (sq(_F) * C2 + _T) + C3

    def _ref(in0, in1, s0, s1, imm2):
        f32 = np.float32
        t = (in0.astype(f32) + f32(s0)).astype(f32)
        u = (t + f32(s1)).astype(f32)
        k = (u - f32(s1)).astype(f32)
        fq = (t - k).astype(f32)
        g = np.asarray(in1, f32).reshape(-1, 1)
        return ((fq * fq) * f32(imm2) + t + g).astype(f32)

    spec = Spec(body=_spill_c3_to_src1(_body), reference=_ref)

    row = dops._CUSTOM_DVE_ROW_BASE + len(dops.OPS)
    assert row < 0x20
    dops._SUB_OPCODE_FOR_NAME[name] = row

    shas = {}
    for ver in ("v3", "v4"):
        try:
            uops = lower(spec, ver=ver)
            shas[ver] = DveOpSpec(
                name=name, opcode=row, uops=uops, rd1_en=True
            ).sha(ver)
        except Exception:
            pass

    op = dops.DveOp(name, spec, subdim=False, uops_sha=shas)
    dops.OPS.append(op)
    dops.CUSTOM_DVE_SPECS[name] = spec
    return op


EXP16_OP = _register_exp16_op()


def _build_program():
    nc = bacc.Bacc("TRN2", target_bir_lowering=False, debug=False,
                   enable_asserts=True, num_devices=N_CORES)

    # ---- per-core DRAM I/O ----
    xt_d = nc.dram_tensor("xt", [128, N], dt.float16, kind="ExternalInput").ap()
    wq0_d = nc.dram_tensor("wq0", [128, 96], dt.float16, kind="ExternalInput").ap()
    wq1_d = nc.dram_tensor("wq1", [128, 96], dt.float16, kind="ExternalInput").ap()
    wk0_d = nc.dram_tensor("wk0", [128, 32], dt.float16, kind="ExternalInput").ap()
    wk1_d = nc.dram_tensor("wk1", [128, 32], dt.float16, kind="ExternalInput").ap()
    wv_d = nc.dram_tensor("wv", [128, 64], dt.float16, kind="ExternalInput").ap()
    wo_d = nc.dram_tensor("wo", [128, 128], dt.float16, kind="ExternalInput").ap()
    # per-head unnormalized projections + row sums (host normalizes)
    y0_d = nc.dram_tensor("y0", [N, 128], dt.float16, kind="ExternalOutput").ap()
    y1_d = nc.dram_tensor("y1", [N, 128], dt.float16, kind="ExternalOutput").ap()
    r_d = nc.dram_tensor("r", [2, N], dt.float16, kind="ExternalOutput").ap()

    ctx = contextlib.ExitStack()
    with tile.TileContext(nc) as tc, ctx:
        # ---- persistent SBUF ----
        per = ctx.enter_context(tc.tile_pool(name="per", bufs=1))
        wq = [per.tile([128, 96], dt.float16, tag=f"wq{h}", name=f"wq{h}")
              for h in range(2)]
        wk = [per.tile([128, 32], dt.float16, tag=f"wk{h}", name=f"wk{h}")
              for h in range(2)]
        wv = per.tile([128, 64], dt.float16)
        wo = per.tile([128, 128], dt.float16)
        # weight + xT DMAs are triggered from the Scalar/GpSimd sequencers:
        # Sync spends ~14us on preamble DIRECT2D writes at kernel start, so
        # DMAs queued there don't even begin until ~9us in. Scalar's preamble
        # is ~2.6us and GpSimd's ~5us; route the early-needed data there.
        nc.scalar.dma_start(wk[0][:], wk0_d[:])
        nc.scalar.dma_start(wq[0][:], wq0_d[:])
        nc.gpsimd.dma_start(wv[:], wv_d[:])
        nc.gpsimd.dma_start(wk[1][:], wk1_d[:])
        nc.gpsimd.dma_start(wq[1][:], wq1_d[:])
        nc.gpsimd.dma_start(wo[:], wo_d[:])
        # xT split into 3 column-chunk tiles (12/12/8 token-tiles)
        XC = (1536, 1536, 1024)
        xt_c = [per.tile([128, XC[ci]], dt.float16, tag=f"xt{ci}",
                         name=f"xt{ci}") for ci in range(3)]
        for ci, eng in enumerate((nc.scalar, nc.gpsimd, nc.sync)):
            eng.dma_start(xt_c[ci][:], xt_d[:, 1536 * ci:1536 * ci + XC[ci]])
        warm = per.tile([1, 8], dt.float32)
        nc.scalar.activation(warm[:], wv[0:1, 0:8], AF.Exp)
        # C3 constant tile for the custom DVE exp
        gconst = per.tile([128, 1], dt.float32, tag="gconst", name="gconst")
        nc.vector.memset(gconst[:], EXP_G)

        # Q^T replicated x3, split into 3 column-chunk tiles per head so
        # scores only RAW-depend on the chunk covering their i-range;
        # K^T block layout [96, 11*128]
        qt = [[per.tile([96, 1536 if q < 2 else 1024], dt.float16,
                        tag=f"qt{h}_{q}", name=f"qt{h}_{q}") for q in range(3)]
              for h in range(2)]
        kt = [[per.tile([96, w], dt.float16, tag=f"kt{h}_{ci}",
                        name=f"kt{h}_{ci}")
               for ci, w in enumerate((512, 512, 384))] for h in range(2)]
        # V_aug for both heads: 4 tiles of 8 j-tiles [128, 8*66] fp16
        # (ones pre-set by memset; split for finer RAW dependencies)
        vsb = [per.tile([128, 8 * VROW], dt.float16, tag=f"v{q}",
                        name=f"vsb{q}") for q in range(4)]
        for q in range(4):
            nc.gpsimd.memset(vsb[q][:], 1.0)

        # ---- PSUM pools ----
        ps_s = ctx.enter_context(tc.tile_pool(name="ps_s", bufs=2, space="PSUM"))
        ps_o = ctx.enter_context(tc.tile_pool(name="ps_o", bufs=2, space="PSUM"))

        sb_p = ctx.enter_context(tc.tile_pool(name="sb_p", bufs=4))
        sb_t = ctx.enter_context(tc.tile_pool(name="sb_t", bufs=2))
        sb_y = ctx.enter_context(tc.tile_pool(name="sb_y", bufs=4))

        # chunk views as [p, token-tile, 128]
        xt3c = [xc.rearrange("p (t jj) -> p t jj", jj=128) for xc in xt_c]

        # ---- prologue projections ----
        def emit_v_round(q):
            pv = ps_s.tile([128, 512], dt.float32, tag="s", name="pv")
            for k in range(8):
                jt = 8 * q + k
                nc.tensor.matmul(pv[:, 64 * k:64 * k + 64],
                                 xt3c[jt // 12][:, jt % 12, :],
                                 wv[:], start=True, stop=True)
            nc.vector.tensor_copy(
                vsb[q][:].rearrange(
                    "p (t a b) -> p t a b", t=8, b=33)[:, :, :, 0:32],
                pv[:].rearrange("p (t a b) -> p t a b", t=8, b=32))

        def emit_kt(h, ci):
            cnt = 4 if ci < 2 else 3
            pk = ps_s.tile([128, 512], dt.float32, tag="s", name="pk")
            for r in range(3):
                c = cnt
                if r == 2 and ci == 2:
                    c = 2  # j-tile 32 doesn't exist (only 0..31)
                rhs = xt3c[ci][:, r:3 * (c - 1) + r + 1:3, :]
                nc.tensor.matmul(pk[32 * r:32 * r + 32, 0:c * 128],
                                 wk[h][:], rhs, start=True, stop=True,
                                 tile_position=(0, 32 * r))
            nc.vector.tensor_copy(kt[h][ci][0:96, :],
                                  pk[0:96, 0:cnt * 128])

        def emit_qt(h, q):
            pq = ps_s.tile([128, 1536], dt.float32, tag="s", name="pq")
            nch = 3 if q < 2 else 2
            for k in range(nch):
                nc.tensor.matmul(pq[0:96, 512 * k:512 * (k + 1)], wq[h][:],
                                 xt_c[q][:, 512 * k:512 * (k + 1)],
                                 start=True, stop=True)
            nc.vector.tensor_copy(qt[h][q][0:96, 0:512 * nch],
                                  pq[0:96, 0:512 * nch])

        # minimal prologue upfront; the rest is interleaved into ic=0's
        # groups (emitted just before the group's scores) so the PE starts
        # the scores/exp pipeline ~15us earlier instead of idling behind
        # DMA-gated projection matmuls.
        emit_kt(0, 0)
        emit_qt(0, 0)
        emit_kt(1, 0)
        emit_qt(1, 0)
        emit_v_round(0)
        prologue_sched = {
            1: [lambda: emit_kt(0, 1), lambda: emit_kt(1, 1)],
            2: [lambda: emit_v_round(1)],
            3: [lambda: emit_kt(0, 2), lambda: emit_kt(1, 2)],
            5: [lambda: emit_qt(0, 1)],
            6: [lambda: emit_v_round(2)],
            7: [lambda: emit_qt(1, 1)],
            8: [lambda: emit_qt(0, 2)],
            9: [lambda: emit_v_round(3)],
            10: [lambda: emit_qt(1, 2)],
        }

        # ---- main loop ----
        groups = [(g, 3) for g in range(10)] + [(10, 2)]

        def emit_proj_h(ic, ot, h):
            # output projection (unnormalized); yh fp16 shipped to host
            pm = ps_s.tile([128, 512], dt.float32, tag="s", name="pm")
            for t4 in range(4):
                nc.tensor.matmul(pm[:, 128 * t4:128 * (t4 + 1)],
                                 ot[64 * h:64 * h + 32,
                                    t4 * 128:(t4 + 1) * 128],
                                 wo[64 * h:64 * h + 32, :],
                                 start=True, stop=True,
                                 tile_position=(64 * h, 0))
            yh = sb_y.tile([128, 512], dt.float16, tag=f"yh{h}",
                           name=f"yh{h}")
            eng = nc.scalar if h == 0 else nc.vector
            if h == 0:
                nc.scalar.activation(yh[:], pm[:], AF.Copy)
            else:
                nc.vector.tensor_copy(yh[:], pm[:])
            yd = y0_d if h == 0 else y1_d
            (nc.sync if h == 0 else nc.gpsimd).dma_start(
                yd[ic * 512:(ic + 1) * 512, :].rearrange(
                    "(t p) c -> p t c", p=128),
                yh[:].rearrange("p (t c) -> p t c", c=128))

        def emit_av(ic, g, nt_, po, pts):
            # AV for both heads, interleaved by j-tile; both heads accumulate
            # into ONE PSUM bank at partition strips 0:33 / 64:97.
            for r in range(nt_):
                jt = 3 * g + r
                for h in range(2):
                    nc.tensor.matmul(
                        po[64 * h:64 * h + 33, :],
                        vsb[jt // 8][:, (jt % 8) * VROW + 33 * h:
                                     (jt % 8) * VROW + 33 * h + 33],
                        pts[h][:, 512 * r:512 * (r + 1)],
                        start=(jt == 0),
                        stop=(jt == NT - 1),
                        tile_position=(0, 64 * h),
                        skip_group_check=True)

        def emit_epilogue(ic, po):
            # one evacuation copy for both heads' out^T strips + sum rows;
            # ship the raw row sums (rows 32 / 96) to DRAM for the host.
            ot = sb_t.tile([128, 512], dt.float16, tag="ot")
            nc.scalar.activation(ot[:], po[:], AF.Copy)
            nc.sync.dma_start(r_d[0:1, ic * 512:(ic + 1) * 512],
                              ot[32:33, :])
            nc.gpsimd.dma_start(r_d[1:2, ic * 512:(ic + 1) * 512],
                                ot[96:97, :])
            return ot

        # exp engine assignment: per (g, h) -> 'S' (ScalarE true exp) or
        # 'V' (DVE custom bit-trick exp). ~12:10 split tuned for balance.
        def exp_engine(g, h):
            if g == 10:
                return 'S' if h == 0 else 'V'
            if h == 0:
                return 'S'
            return 'S' if g == 5 else 'V'

        def emit_exp(eng, pt_ap, ps_ap):
            if eng == 'S':
                nc.scalar.activation(pt_ap, ps_ap, AF.Exp,
                                     scale=1.0 / KAPPA)
            else:
                nc.vector._custom_dve(
                    EXP16_OP, out=pt_ap.bitcast(dt.int16), in0=ps_ap,
                    s0=EXP_H, s1=EXP_M, imm2=EXP_ALPHA, in1=gconst[:])

        # flat software pipeline over (ic, g) steps: scores/exp run TWO
        # groups ahead of AV so the in-order PE stream never stalls waiting
        # for an exp that was issued in the immediately preceding step.
        prev_proj = None
        pend_av = []            # deque of (ic, g, nt_, po, pts), depth 2
        po = None

        def retire_av():
            item = pend_av.pop(0)
            emit_av(*item)
            if item[1] == 10:   # finished that i-chunk's AV
                return (item[0], emit_epilogue(item[0], item[3]))
            return None

        for ic in range(NIC):
            for g, nt_ in groups:
                if ic == 0 and g in prologue_sched:
                    for fn in prologue_sched[g]:
                        fn()
                if g == 0:
                    po = ps_o.tile([128, 512], dt.float32, tag="o",
                                   name="po")
                if g == 3 and prev_proj is not None:
                    emit_proj_h(*prev_proj, 0)
                if g == 6 and prev_proj is not None:
                    emit_proj_h(*prev_proj, 1)
                    prev_proj = None
                pts = []
                for h in range(2):
                    ps = ps_s.tile([128, 1536], dt.float32, tag="s")
                    for r in range(nt_):
                        nc.tensor.matmul(
                            ps[:, 512 * r:512 * (r + 1)],
                            kt[h][g // 4][32 * r:32 * r + 32,
                                          (g % 4) * 128:(g % 4 + 1) * 128],
                            qt[h][ic // 3][32 * r:32 * r + 32,
                                           (ic % 3) * 512:(ic % 3 + 1) * 512],
                            start=True, stop=True, tile_position=(32 * r, 0))
                    pt = sb_p.tile([128, nt_ * 512], dt.float16, tag=f"p{h}")
                    emit_exp(exp_engine(g, h), pt[:], ps[:, 0:nt_ * 512])
                    pts.append(pt)
                depth = 1 if (ic == NIC - 1 and g >= 8) else 2
                while len(pend_av) >= depth:
                    r_ = retire_av()
                    if r_ is not None:
                        prev_proj = r_
                pend_av.append((ic, g, nt_, po, pts))

        while pend_av:
            r_ = retire_av()
            if r_ is not None:
                prev_proj = r_
        emit_proj_h(*prev_proj, 0)
        emit_proj_h(*prev_proj, 1)

    nc.compile()
    return nc


def _host_prep(x, w_qkv, w_out):
    """Build per-core input maps."""
    xf = np.asarray(x, dtype=np.float32).reshape(B, N, C)
    wq_all = np.asarray(w_qkv[:, 0:128], dtype=np.float32)
    wk_all = np.asarray(w_qkv[:, 128:256], dtype=np.float32)
    wv_all = np.asarray(w_qkv[:, 256:384], dtype=np.float32)
    wo_all = np.asarray(w_out, dtype=np.float32)

    xts = [np.ascontiguousarray(xf[b].T).astype(np.float16) for b in range(B)]
    qscale = SCALE * KAPPA

    in_maps = []
    for c in range(N_CORES):
        b = c // 2
        hp = (c % 2) * 2
        wo = np.zeros((128, 128), dtype=np.float16)
        wo[0:32] = wo_all[32 * hp:32 * hp + 32, :]
        wo[64:96] = wo_all[32 * hp + 32:32 * hp + 64, :]
        m = {
            "xt": xts[b],
            "wq0": np.tile(wq_all[:, 32 * hp:32 * hp + 32] * qscale,
                           (1, 3)).astype(np.float16),
            "wq1": np.tile(wq_all[:, 32 * hp + 32:32 * hp + 64] * qscale,
                           (1, 3)).astype(np.float16),
            "wk0": wk_all[:, 32 * hp:32 * hp + 32].astype(np.float16),
            "wk1": wk_all[:, 32 * hp + 32:32 * hp + 64].astype(np.float16),
            "wv": wv_all[:, 32 * hp:32 * hp + 64].astype(np.float16),
            "wo": wo,
        }
        in_maps.append(m)
    return in_maps


def kernel(x, w_qkv, w_out, b_out, _trace=False, _tmpdir=None):
    if "nc" not in _CACHE:
        _CACHE["nc"] = _build_program()
    nc = _CACHE["nc"]

    in_maps = _host_prep(x, w_qkv, w_out)
    res = run_bass_kernel_spmd(nc, in_maps, core_ids=list(range(N_CORES)),
                               trace=_trace, tmpdir=_tmpdir)
    _CACHE["last_result"] = res

    b_out_f = np.asarray(b_out, dtype=np.float32)
    y = np.empty((B, N, C), dtype=np.float32)
    for b in range(B):
        acc = np.zeros((N, C), dtype=np.float32)
        for c in (2 * b, 2 * b + 1):
            rc = res.results[c]
            for h in range(2):
                yh = rc["y0" if h == 0 else "y1"].astype(np.float32)
                rs = rc["r"][h].astype(np.float32)
                acc += yh * (1.0 / rs)[:, None]
        y[b] = acc + b_out_f
    return y.reshape(B, HGT, WID, C)


# revision 28
# speedup vs baseline: 1.1361x; 1.0076x over previous
"""Trainium2 Bass kernel for nn_Attention_72103910965317 (v2).

Multi-head self-attention block (4 heads, head_dim 32, N=4096 tokens/batch,
c=128 channels) over inputs x:[4,64,64,128].

Sharding: 8 cores; core c handles batch c//2 and heads {2*(c%2), 2*(c%2)+1}
(data-parallel over batch x tensor-parallel over heads). Each core computes
per-head attention + its heads' slice of the output projection UNNORMALIZED
(plus the per-head softmax row-sums); the host applies the 1/rowsum
normalization, sums the two per-core partials per batch, and adds b_out.

v2 changes vs v1 (301.9us):
  - The softmax exp (33.5M elems/core, previously 248us of serial ScalarE
    time = the critical path) is SPLIT between ScalarE (true Exp activation)
    and the DVE via a custom 8-stage DVE op (EXP16_BITS_ANT): a floor-based
    Schraudolph bit-trick with quadratic mantissa correction producing fp16
    bits in one 1x-rate pass (max rel err ~0.2%). Both paths emit 0.5*e^s
    (the 0.5 is a bit-trick artifact; softmax normalization cancels it).
  - Scores matmuls feed the bit-trick directly: wq is pre-scaled by
    KAPPA=1024*log2(e) so PSUM holds S' = KAPPA*s; ScalarE path un-scales
    via the activation's free scale/bias slots.
  - Softmax normalization moved to the HOST: the kernel ships per-head
    unnormalized projections (fp16) + row sums; host does
    y = yh0*r0 + yh1*r1 + b_out. Kills the v1 DRAM-bounce/reciprocal/
    per-tile normalize chain on the DVE.
  - AV accumulates both heads into ONE PSUM bank (partition strips 0:33 /
    64:97 via col tile_position), evacuated by a single ScalarE copy.
"""

import os
import sys
import contextlib

for _p in ("/opt/trn_rl_repo", "/root/.axon_site/_ro/trn_rl_repo"):
    if os.path.isdir(_p) and _p not in sys.path:
        sys.path.insert(0, _p)

import numpy as np

import concourse.bass as bass
import concourse.tile as tile
from concourse import bacc, mybir
from concourse.bass_utils import run_bass_kernel_spmd

dt = mybir.dt
AF = mybir.ActivationFunctionType

N_CORES = 8
B, HGT, WID, C = 4, 64, 64, 128
N = HGT * WID          # 4096 tokens per batch
HEADS, D = 4, 32       # heads, head dim
SCALE = D ** -0.5
NT = N // 128          # 32 j-tiles / i-tiles
NIC = N // 512         # 8 i-chunks
VROW = 2 * (D + 1)     # 66: V_aug row for both heads [V_h0|1|V_h1|1]

# ---- custom DVE exp constants (see fit in problem notes) ----
KAPPA = 1024.0 * np.log2(np.e)          # 1477.3199...
EXP_H = 14839.92186188657               # s0: octave-alignment offset
EXP_M = 1.5 * 2.0 ** 33                 # s1: round-to-1024 magic
EXP_ALPHA = 0.0003292511551447068       # imm2: quadratic mantissa coeff
EXP_G = -591.0718312168698 + 1024.0     # C3 (via in1): bits bias (+1024
                                        # doubles the bit-trick output to
                                        # 1.0*e^s, matching plain ScalarE exp)

_CACHE = {}


def _register_exp16_op():
    """Define + register the EXP16_BITS_ANT custom DVE op (idempotent).

    bits = sq(F)*alpha + T + g, T = Src0 + H, F = T - ((T+M)-M)
    out(int16) bit-cast to fp16 gives 0.5*exp(Src0/KAPPA) to ~0.2%.
    """
    from concourse import dve_ops as dops
    from concourse.dve_spec import (
        Spec, Src0, C0, C1, C2, C3, _spill_c3_to_src1, sq, lower,
    )
    from concourse.dve_uop import DveOpSpec

    name = "EXP16_BITS_ANT"
    if name in dops._SUB_OPCODE_FOR_NAME:
        return next(op for op in dops.OPS if op.name == name)

    _T = Src0 + C0
    _u = _T + C1
    _K = _u - C1
    _F = _T - _K
    _body = (sq(_F) * C2 + _T) + C3

    def _ref(in0, in1, s0, s1, imm2):
        f32 = np.float32
        t = (in0.astype(f32) + f32(s0)).astype(f32)
        u = (t + f32(s1)).astype(f32)
        k = (u - f32(s1)).astype(f32)
        fq = (t - k).astype(f32)
        g = np.asarray(in1, f32).reshape(-1, 1)
        return ((fq * fq) * f32(imm2) + t + g).astype(f32)

    spec = Spec(body=_spill_c3_to_src1(_body), reference=_ref)

    row = dops._CUSTOM_DVE_ROW_BASE + len(dops.OPS)
    assert row < 0x20
    dops._SUB_OPCODE_FOR_NAME[name] = row

    shas = {}
    for ver in ("v3", "v4"):
        try:
            uops = lower(spec, ver=ver)
            shas[ver] = DveOpSpec(
                name=name, opcode=row, uops=uops, rd1_en=True
            ).sha(ver)
        except Exception:
            pass

    op = dops.DveOp(name, spec, subdim=False, uops_sha=shas)
    dops.OPS.append(op)
    dops.CUSTOM_DVE_SPECS[name] = spec
    return op


EXP16_OP = _register_exp16_op()


def _build_program():
    nc = bacc.Bacc("TRN2", target_bir_lowering=False, debug=False,
                   enable_asserts=True, num_devices=N_CORES)

    # ---- per-core DRAM I/O ----
    xt_d = nc.dram_tensor("xt", [128, N], dt.float16, kind="ExternalInput").ap()
    wq0_d = nc.dram_tensor("wq0", [128, 96], dt.float16, kind="ExternalInput").ap()
    wq1_d = nc.dram_tensor("wq1", [128, 96], dt.float16, kind="ExternalInput").ap()
    wk0_d = nc.dram_tensor("wk0", [128, 32], dt.float16, kind="ExternalInput").ap()
    wk1_d = nc.dram_tensor("wk1", [128, 32], dt.float16, kind="ExternalInput").ap()
    wv_d = nc.dram_tensor("wv", [128, 64], dt.float16, kind="ExternalInput").ap()
    wo_d = nc.dram_tensor("wo", [128, 128], dt.float16, kind="ExternalInput").ap()
    # per-head unnormalized projections + row sums (host normalizes)
    y0_d = nc.dram_tensor("y0", [N, 128], dt.float16, kind="ExternalOutput").ap()
    y1_d = nc.dram_tensor("y1", [N, 128], dt.float16, kind="ExternalOutput").ap()
    r_d = nc.dram_tensor("r", [2, N], dt.float16, kind="ExternalOutput").ap()

    ctx = contextlib.ExitStack()
    with tile.TileContext(nc) as tc, ctx:
        # ---- persistent SBUF ----
        per = ctx.enter_context(tc.tile_pool(name="per", bufs=1))
        wq = [per.tile([128, 96], dt.float16, tag=f"wq{h}", name=f"wq{h}")
              for h in range(2)]
        wk = [per.tile([128, 32], dt.float16, tag=f"wk{h}", name=f"wk{h}")
              for h in range(2)]
        wv = per.tile([128, 64], dt.float16)
        wo = per.tile([128, 128], dt.float16)
        # weight + xT DMAs are triggered from the Scalar/GpSimd sequencers:
        # Sync spends ~14us on preamble DIRECT2D writes at kernel start, so
        # DMAs queued there don't even begin until ~9us in. Scalar's preamble
        # is ~2.6us and GpSimd's ~5us; route the early-needed data there.
        nc.scalar.dma_start(wk[0][:], wk0_d[:])
        nc.scalar.dma_start(wq[0][:], wq0_d[:])
        nc.gpsimd.dma_start(wv[:], wv_d[:])
        nc.gpsimd.dma_start(wk[1][:], wk1_d[:])
        nc.gpsimd.dma_start(wq[1][:], wq1_d[:])
        nc.gpsimd.dma_start(wo[:], wo_d[:])
        # xT split into 3 column-chunk tiles (12/12/8 token-tiles)
        XC = (1536, 1536, 1024)
        xt_c = [per.tile([128, XC[ci]], dt.float16, tag=f"xt{ci}",
                         name=f"xt{ci}") for ci in range(3)]
        for ci, eng in enumerate((nc.scalar, nc.gpsimd, nc.sync)):
            eng.dma_start(xt_c[ci][:], xt_d[:, 1536 * ci:1536 * ci + XC[ci]])
        warm = per.tile([1, 8], dt.float32)
        nc.scalar.activation(warm[:], wv[0:1, 0:8], AF.Exp)
        # C3 constant tile for the custom DVE exp
        gconst = per.tile([128, 1], dt.float32, tag="gconst", name="gconst")
        nc.vector.memset(gconst[:], EXP_G)

        # Q^T replicated x3, split into 3 column-chunk tiles per head so
        # scores only RAW-depend on the chunk covering their i-range;
        # K^T block layout [96, 11*128]
        qt = [[per.tile([96, 1536 if q < 2 else 1024], dt.float16,
                        tag=f"qt{h}_{q}", name=f"qt{h}_{q}") for q in range(3)]
              for h in range(2)]
        kt = [[per.tile([96, w], dt.float16, tag=f"kt{h}_{ci}",
                        name=f"kt{h}_{ci}")
               for ci, w in enumerate((512, 512, 384))] for h in range(2)]
        # V_aug for both heads: 4 tiles of 8 j-tiles [128, 8*66] fp16
        # (ones pre-set by memset; split for finer RAW dependencies)
        vsb = [per.tile([128, 8 * VROW], dt.float16, tag=f"v{q}",
                        name=f"vsb{q}") for q in range(4)]
        for q in range(4):
            nc.gpsimd.memset(vsb[q][:], 1.0)

        # ---- PSUM pools ----
        ps_s = ctx.enter_context(tc.tile_pool(name="ps_s", bufs=2, space="PSUM"))
        ps_o = ctx.enter_context(tc.tile_pool(name="ps_o", bufs=2, space="PSUM"))

        sb_p = ctx.enter_context(tc.tile_pool(name="sb_p", bufs=4))
        sb_t = ctx.enter_context(tc.tile_pool(name="sb_t", bufs=2))
        sb_y = ctx.enter_context(tc.tile_pool(name="sb_y", bufs=4))

        # chunk views as [p, token-tile, 128]
        xt3c = [xc.rearrange("p (t jj) -> p t jj", jj=128) for xc in xt_c]

        # ---- prologue projections ----
        def emit_v_round(q):
            pv = ps_s.tile([128, 512], dt.float32, tag="s", name="pv")
            for k in range(8):
                jt = 8 * q + k
                nc.tensor.matmul(pv[:, 64 * k:64 * k + 64],
                                 xt3c[jt // 12][:, jt % 12, :],
                                 wv[:], start=True, stop=True)
            nc.vector.tensor_copy(
                vsb[q][:].rearrange(
                    "p (t a b) -> p t a b", t=8, b=33)[:, :, :, 0:32],
                pv[:].rearrange("p (t a b) -> p t a b", t=8, b=32))

        def emit_kt(h, ci):
            cnt = 4 if ci < 2 else 3
            pk = ps_s.tile([128, 512], dt.float32, tag="s", name="pk")
            for r in range(3):
                c = cnt
                if r == 2 and ci == 2:
                    c = 2  # j-tile 32 doesn't exist (only 0..31)
                rhs = xt3c[ci][:, r:3 * (c - 1) + r + 1:3, :]
                nc.tensor.matmul(pk[32 * r:32 * r + 32, 0:c * 128],
                                 wk[h][:], rhs, start=True, stop=True,
                                 tile_position=(0, 32 * r))
            nc.vector.tensor_copy(kt[h][ci][0:96, :],
                                  pk[0:96, 0:cnt * 128])

        def emit_qt(h, q):
            pq = ps_s.tile([128, 1536], dt.float32, tag="s", name="pq")
            nch = 3 if q < 2 else 2
            for k in range(nch):
                nc.tensor.matmul(pq[0:96, 512 * k:512 * (k + 1)], wq[h][:],
                                 xt_c[q][:, 512 * k:512 * (k + 1)],
                                 start=True, stop=True)
            nc.vector.tensor_copy(qt[h][q][0:96, 0:512 * nch],
                                  pq[0:96, 0:512 * nch])

        # minimal prologue upfront; the rest is interleaved into ic=0's
        # groups (emitted just before the group's scores) so the PE starts
        # the scores/exp pipeline ~15us earlier instead of idling behind
        # DMA-gated projection matmuls.
        emit_kt(0, 0)
        emit_qt(0, 0)
        emit_kt(1, 0)
        emit_qt(1, 0)
        emit_v_round(0)
        prologue_sched = {
            1: [lambda: emit_kt(0, 1), lambda: emit_kt(1, 1)],
            2: [lambda: emit_v_round(1)],
            3: [lambda: emit_kt(0, 2), lambda: emit_kt(1, 2)],
            5: [lambda: emit_qt(0, 1)],
            6: [lambda: emit_v_round(2)],
            7: [lambda: emit_qt(1, 1)],
            8: [lambda: emit_qt(0, 2)],
            9: [lambda: emit_v_round(3)],
            10: [lambda: emit_qt(1, 2)],
        }

        # ---- main loop ----
        groups = [(g, 3) for g in range(10)] + [(10, 2)]

        def emit_proj_h(ic, ot, h):
            # output projection (unnormalized); yh fp16 shipped to host
            pm = ps_s.tile([128, 512], dt.float32, tag="s", name="pm")
            for t4 in range(4):
                nc.tensor.matmul(pm[:, 128 * t4:128 * (t4 + 1)],
                                 ot[64 * h:64 * h + 32,
                                    t4 * 128:(t4 + 1) * 128],
                                 wo[64 * h:64 * h + 32, :],
                                 start=True, stop=True,
                                 tile_position=(64 * h, 0))
            yh = sb_y.tile([128, 512], dt.float16, tag=f"yh{h}",
                           name=f"yh{h}")
            eng = nc.scalar if h == 0 else nc.vector
            if h == 0:
                nc.scalar.activation(yh[:], pm[:], AF.Copy)
            else:
                nc.vector.tensor_copy(yh[:], pm[:])
            yd = y0_d if h == 0 else y1_d
            (nc.sync if h == 0 else nc.gpsimd).dma_start(
                yd[ic * 512:(ic + 1) * 512, :].rearrange(
                    "(t p) c -> p t c", p=128),
                yh[:].rearrange("p (t c) -> p t c", c=128))

        def emit_av(ic, g, nt_, po, pts):
            # AV for both heads, interleaved by j-tile; both heads accumulate
            # into ONE PSUM bank at partition strips 0:33 / 64:97.
            for r in range(nt_):
                jt = 3 * g + r
                for h in range(2):
                    nc.tensor.matmul(
                        po[64 * h:64 * h + 33, :],
                        vsb[jt // 8][:, (jt % 8) * VROW + 33 * h:
                                     (jt % 8) * VROW + 33 * h + 33],
                        pts[h][:, 512 * r:512 * (r + 1)],
                        start=(jt == 0),
                        stop=(jt == NT - 1),
                        tile_position=(0, 64 * h),
                        skip_group_check=True)

        def emit_epilogue(ic, po):
            # one evacuation copy for both heads' out^T strips + sum rows;
            # ship the raw row sums (rows 32 / 96) to DRAM for the host.
            ot = sb_t.tile([128, 512], dt.float16, tag="ot")
            nc.scalar.activation(ot[:], po[:], AF.Copy)
            nc.sync.dma_start(r_d[0:1, ic * 512:(ic + 1) * 512],
                              ot[32:33, :])
            nc.gpsimd.dma_start(r_d[1:2, ic * 512:(ic + 1) * 512],
                                ot[96:97, :])
            return ot

        # exp engine assignment: per (g, h) -> 'S' (ScalarE true exp) or
        # 'V' (DVE custom bit-trick exp). ~12:10 split tuned for balance.
        def exp_engine(g, h):
            if g == 10:
                return 'S' if h == 0 else 'V'
            if h == 0:
                return 'S'
            return 'S' if g == 5 else 'V'

        def emit_exp(eng, pt_ap, ps_ap):
            if eng == 'S':
                nc.scalar.activation(pt_ap, ps_ap, AF.Exp,
                                     scale=1.0 / KAPPA)
            else:
                nc.vector._custom_dve(
                    EXP16_OP, out=pt_ap.bitcast(dt.int16), in0=ps_ap,
                    s0=EXP_H, s1=EXP_M, imm2=EXP_ALPHA, in1=gconst[:])

        # flat software pipeline over (ic, g) steps: scores/exp run TWO
        # groups ahead of AV so the in-order PE stream never stalls waiting
        # for an exp that was issued in the immediately preceding step.
        prev_proj = None
        pend_av = []            # deque of (ic, g, nt_, po, pts), depth 2
        po = None

        def retire_av():
            item = pend_av.pop(0)
            emit_av(*item)
            if item[1] == 10:   # finished that i-chunk's AV
                return (item[0], emit_epilogue(item[0], item[3]))
            return None

        for ic in range(NIC):
            for g, nt_ in groups:
                if ic == 0 and g in prologue_sched:
                    for fn in prologue_sched[g]:
                        fn()
                if g == 0:
                    po = ps_o.tile([128, 512], dt.float32, tag="o",
                                   name="po")
                if g == 3 and prev_proj is not None:
                    emit_proj_h(*prev_proj, 0)
                if g == 6 and prev_proj is not None:
                    emit_proj_h(*prev_proj, 1)
                    prev_proj = None
                pts = []
                for h in range(2):
                    ps = ps_s.tile([128, 1536], dt.float32, tag="s")
                    for r in range(nt_):
                        nc.tensor.matmul(
                            ps[:, 512 * r:512 * (r + 1)],
                            kt[h][g // 4][32 * r:32 * r + 32,
                                          (g % 4) * 128:(g % 4 + 1) * 128],
                            qt[h][ic // 3][32 * r:32 * r + 32,
                                           (ic % 3) * 512:(ic % 3 + 1) * 512],
                            start=True, stop=True, tile_position=(32 * r, 0))
                    pt = sb_p.tile([128, nt_ * 512], dt.float16, tag=f"p{h}")
                    emit_exp(exp_engine(g, h), pt[:], ps[:, 0:nt_ * 512])
                    pts.append(pt)
                depth = 1 if (ic == NIC - 1 and g >= 8) else 3
                while len(pend_av) >= depth:
                    r_ = retire_av()
                    if r_ is not None:
                        prev_proj = r_
                pend_av.append((ic, g, nt_, po, pts))

        while pend_av:
            r_ = retire_av()
            if r_ is not None:
                prev_proj = r_
        emit_proj_h(*prev_proj, 0)
        emit_proj_h(*prev_proj, 1)

    nc.compile()
    return nc


def _host_prep(x, w_qkv, w_out):
    """Build per-core input maps."""
    xf = np.asarray(x, dtype=np.float32).reshape(B, N, C)
    wq_all = np.asarray(w_qkv[:, 0:128], dtype=np.float32)
    wk_all = np.asarray(w_qkv[:, 128:256], dtype=np.float32)
    wv_all = np.asarray(w_qkv[:, 256:384], dtype=np.float32)
    wo_all = np.asarray(w_out, dtype=np.float32)

    xts = [np.ascontiguousarray(xf[b].T).astype(np.float16) for b in range(B)]
    qscale = SCALE * KAPPA

    in_maps = []
    for c in range(N_CORES):
        b = c // 2
        hp = (c % 2) * 2
        wo = np.zeros((128, 128), dtype=np.float16)
        wo[0:32] = wo_all[32 * hp:32 * hp + 32, :]
        wo[64:96] = wo_all[32 * hp + 32:32 * hp + 64, :]
        m = {
            "xt": xts[b],
            "wq0": np.tile(wq_all[:, 32 * hp:32 * hp + 32] * qscale,
                           (1, 3)).astype(np.float16),
            "wq1": np.tile(wq_all[:, 32 * hp + 32:32 * hp + 64] * qscale,
                           (1, 3)).astype(np.float16),
            "wk0": wk_all[:, 32 * hp:32 * hp + 32].astype(np.float16),
            "wk1": wk_all[:, 32 * hp + 32:32 * hp + 64].astype(np.float16),
            "wv": wv_all[:, 32 * hp:32 * hp + 64].astype(np.float16),
            "wo": wo,
        }
        in_maps.append(m)
    return in_maps


def kernel(x, w_qkv, w_out, b_out, _trace=False, _tmpdir=None):
    if "nc" not in _CACHE:
        _CACHE["nc"] = _build_program()
    nc = _CACHE["nc"]

    in_maps = _host_prep(x, w_qkv, w_out)
    res = run_bass_kernel_spmd(nc, in_maps, core_ids=list(range(N_CORES)),
                               trace=_trace, tmpdir=_tmpdir)
    _CACHE["last_result"] = res

    b_out_f = np.asarray(b_out, dtype=np.float32)
    y = np.empty((B, N, C), dtype=np.float32)
    for b in range(B):
        acc = np.zeros((N, C), dtype=np.float32)
        for c in (2 * b, 2 * b + 1):
            rc = res.results[c]
            for h in range(2):
                yh = rc["y0" if h == 0 else "y1"].astype(np.float32)
                rs = rc["r"][h].astype(np.float32)
                acc += yh * (1.0 / rs)[:, None]
        y[b] = acc + b_out_f
    return y.reshape(B, HGT, WID, C)
